# revision 7
# baseline (speedup 1.0000x reference)
"""Bidirectional LSTM layer on 8 Trainium2 NeuronCores.

Strategy (spec: T=256, B=128, IN=512, H=1024):
  - 8-way tensor-parallel over the 4H gate dim: core j owns gate rows
    {g*1024 + j*128 .. +128} for g in (f, i, g, o) [reordered so one
    sigmoid covers f+i], i.e. a 512-wide gate slice, and therefore owns
    h/c columns j*128..(j+1)*128.
  - Per timestep: PSUM[b=128, 512] = bias (K=1 matmul) + x_t^T-chunks @
    Wih^T-slice + h_t^T-chunks @ Whh^T-slice, all f32r (full-rate fp32).
    Activations read PSUM; cell update on DVE; h chunk is PE-transposed
    and AllGathered (per direction) into the next step's stationary h^T.
  - fwd and rev directions are independent chains, staggered so PE works
    on one while the other's AllGather is in flight.
  - X^T is distributed once at start: each core transposes nothing; the
    host passes its T-slice of X^T and one AllGather replicates it.
"""
import sys
sys.path.insert(0, "/opt/trn_rl_repo")
import numpy as np
import concourse.bacc as bacc
import concourse.mybir as mybir
from concourse import tile
from concourse import bass_utils

T, B, IN, H = 256, 128, 512, 1024
NC = 8
TS = T // NC            # timesteps per core in the X^T shard
F32R = mybir.dt.float32r
F32 = mybir.dt.float32
AF = mybir.ActivationFunctionType

_cache = {}


def build(reps=1, t_steps=T):
    nc = bacc.Bacc("TRN2", target_bir_lowering=False, debug=False, num_devices=NC)

    xt_in = nc.dram_tensor("xt", [IN, TS * B], F32R, kind="ExternalInput")
    dirs = ("f", "r")
    whh_in = {d: nc.dram_tensor(f"whh_{d}", [H, 512], F32R, kind="ExternalInput") for d in dirs}
    wih_in = {d: nc.dram_tensor(f"wih_{d}", [IN, 512], F32R, kind="ExternalInput") for d in dirs}
    bias_in = {d: nc.dram_tensor(f"bias_{d}", [1, 512], F32R, kind="ExternalInput") for d in dirs}
    h0t_in = {d: nc.dram_tensor(f"h0t_{d}", [128, H], F32R, kind="ExternalInput") for d in dirs}
    c0_in = {d: nc.dram_tensor(f"c0_{d}", [128, 128], F32R, kind="ExternalInput") for d in dirs}
    ones_in = nc.dram_tensor("ones", [1, 128], F32R, kind="ExternalInput")
    ident_in = nc.dram_tensor("ident", [128, 128], F32R, kind="ExternalInput")

    y_out = nc.dram_tensor("y", [T, B, 256], F32R, kind="ExternalOutput")
    state_out = nc.dram_tensor("state", [4, 128, 128], F32R, kind="ExternalOutput")

    with tile.TileContext(nc) as tc:
        with (
            tc.tile_pool(name="const", bufs=1) as cpool,
            tc.tile_pool(name="xts", bufs=6) as xpool,
            tc.tile_pool(name="ht", bufs=4) as hpool,
            tc.tile_pool(name="gates", bufs=4) as gpool,
            tc.tile_pool(name="small", bufs=4) as spool,
            tc.tile_pool(name="gpsum", bufs=3, space="PSUM") as pspool,
            tc.tile_pool(name="tpsum", bufs=2, space="PSUM") as tpool,
            tc.tile_pool(name="dram", bufs=1, space="DRAM") as dpool,
            tc.tile_pool(name="dbounce", bufs=8, space="DRAM") as dbpool,
        ):
            # ---- Phase 0: weights + X^T distribution ----
            whh_sb, wih_sb, bias_sb = {}, {}, {}
            for d in dirs:
                whh_sb[d] = cpool.tile([128, 8 * 512], F32R, tag=f"whh{d}", name=f"whh_sb_{d}")
                nc.sync.dma_start(
                    whh_sb[d][:].rearrange("p (c n) -> p c n", n=512),
                    whh_in[d][:].rearrange("(c p) n -> p c n", p=128),
                )
                wih_sb[d] = cpool.tile([128, 4 * 512], F32R, tag=f"wih{d}", name=f"wih_sb_{d}")
                nc.sync.dma_start(
                    wih_sb[d][:].rearrange("p (c n) -> p c n", n=512),
                    wih_in[d][:].rearrange("(c p) n -> p c n", p=128),
                )
                bias_sb[d] = cpool.tile([1, 512], F32R, tag=f"bias{d}", name=f"bias_sb_{d}")
                nc.sync.dma_start(bias_sb[d][:], bias_in[d][:])
            ones_sb = cpool.tile([1, 128], F32R, tag="ones")
            nc.sync.dma_start(ones_sb[:], ones_in[:])
            ident_sb = cpool.tile([128, 128], F32R, tag="ident")
            nc.sync.dma_start(ident_sb[:], ident_in[:])

            xt_bounce = dpool.tile([IN, TS * B], F32R, tag="xtb")
            nc.sync.dma_start(xt_bounce[:], xt_in[:])
            xt_full = dpool.tile([NC * IN, TS * B], F32R, tag="xtf")
            nc.gpsimd.collective_compute(
                "AllGather", mybir.AluOpType.bypass,
                replica_groups=[list(range(NC))],
                ins=[xt_bounce.opt()], outs=[xt_full.opt()],
            )

            ht = {}
            ct = {}
            for d in dirs:
                ht[d] = hpool.tile([128, H], F32R, tag=f"ht{d}", name=f"ht0_{d}")
                nc.sync.dma_start(ht[d][:], h0t_in[d][:])
                ct[d] = spool.tile([128, 128], F32R, tag=f"c{d}", name=f"ct0_{d}")
                nc.sync.dma_start(ct[d][:], c0_in[d][:])

            # ---- Recurrence ----
            hn_last = {}
            for rep in range(reps):
                for t in range(t_steps):
                    for d in dirs:
                        td = t if d == "f" else (T - 1 - t)
                        r, tl = divmod(td, TS)

                        xts = xpool.tile([128, 512], F32R, tag=f"x{d}")
                        nc.sync.dma_start(
                            xts[:].rearrange("p (c b) -> p c b", b=128),
                            xt_full[r * IN:(r + 1) * IN, tl * B:(tl + 1) * B]
                            .rearrange("(c p) b -> p c b", p=128),
                        )

                        # gate order [f i | o g]; psum split in two banks so
                        # activations start after half the matmuls
                        psa = pspool.tile([128, 256], F32, tag="psa")
                        psb = pspool.tile([128, 256], F32, tag="psb")
                        nc.tensor.matmul(psa[:], ones_sb[0:1, :], bias_sb[d][0:1, 0:256],
                                         start=True, stop=False)
                        for c in range(4):
                            nc.tensor.matmul(
                                psa[:], xts[:, c * 128:(c + 1) * 128],
                                wih_sb[d][:, c * 512:c * 512 + 256],
                                start=False, stop=False)
                        for c in range(8):
                            nc.tensor.matmul(
                                psa[:], ht[d][:, c * 128:(c + 1) * 128],
                                whh_sb[d][:, c * 512:c * 512 + 256],
                                start=False, stop=(c == 7))
                        nc.tensor.matmul(psb[:], ones_sb[0:1, :], bias_sb[d][0:1, 256:512],
                                         start=True, stop=False)
                        for c in range(4):
                            nc.tensor.matmul(
                                psb[:], xts[:, c * 128:(c + 1) * 128],
                                wih_sb[d][:, c * 512 + 256:(c + 1) * 512],
                                start=False, stop=False)
                        for c in range(8):
                            nc.tensor.matmul(
                                psb[:], ht[d][:, c * 128:(c + 1) * 128],
                                whh_sb[d][:, c * 512 + 256:(c + 1) * 512],
                                start=False, stop=(c == 7))

                        # g layout [f i o g]
                        g = gpool.tile([128, 512], F32R, tag=f"g{d}")
                        nc.scalar.activation(g[:, 0:256], psa[:, 0:256], AF.Sigmoid)
                        nc.scalar.activation(g[:, 384:512], psb[:, 128:256], AF.Tanh)
                        nc.scalar.activation(g[:, 256:384], psb[:, 0:128], AF.Sigmoid)

                        t1 = spool.tile([128, 128], F32R, tag=f"t1{d}")
                        nc.vector.tensor_mul(t1[:], g[:, 0:128], ct[d][:])
                        t2 = spool.tile([128, 128], F32R, tag=f"t2{d}")
                        nc.vector.tensor_mul(t2[:], g[:, 128:256], g[:, 384:512])
                        cn = spool.tile([128, 128], F32R, tag=f"c{d}")
                        nc.vector.tensor_add(cn[:], t1[:], t2[:])
                        th = spool.tile([128, 128], F32R, tag=f"th{d}")
                        nc.scalar.activation(th[:], cn[:], AF.Tanh)
                        hn = spool.tile([128, 128], F32R, tag=f"hn{d}")
                        nc.vector.tensor_mul(hn[:], g[:, 256:384], th[:])
                        ct[d] = cn

                        # y[t, :, dir*128 : +128] <- h chunk
                        off = 0 if d == "f" else 128
                        nc.sync.dma_start(y_out[td, :, off:off + 128], hn[:])

                        # transpose h chunk, allgather into next h^T
                        tp = tpool.tile([128, 128], F32R, tag="tp")
                        nc.tensor.transpose(tp[:], hn[:], ident_sb[:])
                        htc = spool.tile([128, 128], F32R, tag=f"htc{d}")
                        nc.vector.tensor_copy(htc[:], tp[:])
                        hbin = dbpool.tile([128, 128], F32R, tag=f"hbin{d}")
                        nc.sync.dma_start(hbin[:], htc[:])
                        hbout = dbpool.tile([NC * 128, 128], F32R, tag=f"hbout{d}")
                        nc.gpsimd.collective_compute(
                            "AllGather", mybir.AluOpType.bypass,
                            replica_groups=[list(range(NC))],
                            ins=[hbin.opt()], outs=[hbout.opt()],
                        )
                        htn = hpool.tile([128, H], F32R, tag=f"ht{d}")
                        htn_v = htn[:].rearrange("p (c b) -> p c b", b=128)
                        hbo_v = hbout[:].rearrange("(c p) b -> p c b", p=128)
                        nc.sync.dma_start(htn_v[:, 0:4, :], hbo_v[:, 0:4, :])
                        nc.sync.dma_start(htn_v[:, 4:8, :], hbo_v[:, 4:8, :])
                        ht[d] = htn
                        hn_last[d] = hn

                # final states: fwd last step / rev last processed (t=0)
                nc.sync.dma_start(state_out[0], hn_last["f"][:])
                nc.sync.dma_start(state_out[1], ct["f"][:])
                nc.sync.dma_start(state_out[2], hn_last["r"][:])
                nc.sync.dma_start(state_out[3], ct["r"][:])

    nc.compile()
    return nc


def _prep_inputs(x, h0_fwd, c0_fwd, h0_rev, c0_rev,
                 W_ih_f, W_hh_f, b_ih_f, b_hh_f,
                 W_ih_r, W_hh_r, b_ih_r, b_hh_r):
    f32 = np.float32
    x = np.ascontiguousarray(x, f32)
    xt_all = np.ascontiguousarray(x.reshape(T * B, IN).T)  # [IN, T*B]
    per_dir = {
        "f": (W_ih_f, W_hh_f, b_ih_f, b_hh_f, h0_fwd, c0_fwd),
        "r": (W_ih_r, W_hh_r, b_ih_r, b_hh_r, h0_rev, c0_rev),
    }
    ones = np.ones((1, 128), f32)
    ident = np.eye(128, dtype=f32)
    in_maps = []
    for j in range(NC):
        idx = np.concatenate([
            np.arange(128) + 1024 + j * 128,   # f
            np.arange(128) + 0 + j * 128,      # i
            np.arange(128) + 3072 + j * 128,   # o
            np.arange(128) + 2048 + j * 128,   # g
        ])
        m = {
            "xt": np.ascontiguousarray(xt_all[:, j * TS * B:(j + 1) * TS * B]),
            "ones": ones, "ident": ident,
        }
        for d in dirs_tuple():
            Wih, Whh, bih, bhh, h0, c0 = per_dir[d]
            m[f"whh_{d}"] = np.ascontiguousarray(np.asarray(Whh, f32)[idx, :].T)
            m[f"wih_{d}"] = np.ascontiguousarray(np.asarray(Wih, f32)[idx, :].T)
            m[f"bias_{d}"] = np.ascontiguousarray(
                (np.asarray(bih, f32) + np.asarray(bhh, f32))[idx][None, :])
            m[f"h0t_{d}"] = np.ascontiguousarray(
                np.asarray(h0, f32).reshape(B, 8, 128).transpose(2, 1, 0).reshape(128, H))
            m[f"c0_{d}"] = np.ascontiguousarray(np.asarray(c0, f32)[:, j * 128:(j + 1) * 128])
        in_maps.append(m)
    return in_maps


def dirs_tuple():
    return ("f", "r")


def run(inputs, reps=1):
    key = reps
    if key not in _cache:
        _cache[key] = build(reps)
    nc = _cache[key]
    in_maps = _prep_inputs(**inputs)
    res = bass_utils.run_bass_kernel_spmd(
        nc, in_maps, core_ids=list(range(NC)), trace=False)
    return res.results


def kernel(**inputs):
    results = run(inputs, reps=1)
    f32 = np.float32
    y = np.empty((T, B, 2 * H), f32)
    hf = np.empty((B, H), f32)
    cf = np.empty((B, H), f32)
    hr = np.empty((B, H), f32)
    cr = np.empty((B, H), f32)
    for j in range(NC):
        yj = results[j]["y"]
        y[:, :, j * 128:(j + 1) * 128] = yj[:, :, 0:128]
        y[:, :, H + j * 128:H + (j + 1) * 128] = yj[:, :, 128:256]
        st = results[j]["state"]
        hf[:, j * 128:(j + 1) * 128] = st[0]
        cf[:, j * 128:(j + 1) * 128] = st[1]
        hr[:, j * 128:(j + 1) * 128] = st[2]
        cr[:, j * 128:(j + 1) * 128] = st[3]
    hidden = np.stack([hf, cf])
    cell = np.stack([hr, cr])
    return y, (hidden, cell)


# revision 8
# speedup vs baseline: 6.9318x; 6.9318x over previous
"""Bidirectional LSTM layer on 8 Trainium2 NeuronCores.

Strategy (spec: T=256, B=128, IN=512, H=1024):
  - 8-way tensor-parallel over the 4H gate dim: core j owns gate rows
    {g*1024 + j*128 .. +128} for g in (f, i, g, o) [reordered so one
    sigmoid covers f+i], i.e. a 512-wide gate slice, and therefore owns
    h/c columns j*128..(j+1)*128.
  - Per timestep: PSUM[b=128, 512] = bias (K=1 matmul) + x_t^T-chunks @
    Wih^T-slice + h_t^T-chunks @ Whh^T-slice, all f32r (full-rate fp32).
    Activations read PSUM; cell update on DVE; h chunk is PE-transposed
    and AllGathered (per direction) into the next step's stationary h^T.
  - fwd and rev directions are independent chains, staggered so PE works
    on one while the other's AllGather is in flight.
  - X^T is distributed once at start: each core transposes nothing; the
    host passes its T-slice of X^T and one AllGather replicates it.
"""
import sys
sys.path.insert(0, "/opt/trn_rl_repo")
import numpy as np
import concourse.bacc as bacc
import concourse.mybir as mybir
from concourse import tile
from concourse import bass_utils

T, B, IN, H = 256, 128, 512, 1024
NC = 8
TS = T // NC            # timesteps per core in the X^T shard
F32R = mybir.dt.float32r
F32 = mybir.dt.float32
AF = mybir.ActivationFunctionType

_cache = {}


def build(reps=1, t_steps=T):
    nc = bacc.Bacc("TRN2", target_bir_lowering=False, debug=False, num_devices=NC)

    xt_in = nc.dram_tensor("xt", [IN, TS * B], F32R, kind="ExternalInput")
    dirs = ("f", "r")
    whh_in = {d: nc.dram_tensor(f"whh_{d}", [H, 512], F32R, kind="ExternalInput") for d in dirs}
    wih_in = {d: nc.dram_tensor(f"wih_{d}", [IN, 512], F32R, kind="ExternalInput") for d in dirs}
    bias_in = {d: nc.dram_tensor(f"bias_{d}", [1, 512], F32R, kind="ExternalInput") for d in dirs}
    h0t_in = {d: nc.dram_tensor(f"h0t_{d}", [128, H], F32R, kind="ExternalInput") for d in dirs}
    c0_in = {d: nc.dram_tensor(f"c0_{d}", [128, 128], F32R, kind="ExternalInput") for d in dirs}
    ones_in = nc.dram_tensor("ones", [1, 128], F32R, kind="ExternalInput")
    ident_in = nc.dram_tensor("ident", [128, 128], F32R, kind="ExternalInput")

    y_out = nc.dram_tensor("y", [T, B, 256], F32R, kind="ExternalOutput")
    state_out = nc.dram_tensor("state", [4, 128, 128], F32R, kind="ExternalOutput")

    with tile.TileContext(nc) as tc:
        with (
            tc.tile_pool(name="const", bufs=1) as cpool,
            tc.tile_pool(name="xts", bufs=6) as xpool,
            tc.tile_pool(name="ht", bufs=4) as hpool,
            tc.tile_pool(name="gates", bufs=4) as gpool,
            tc.tile_pool(name="small", bufs=4) as spool,
            tc.tile_pool(name="gpsum", bufs=4, space="PSUM") as pspool,
            tc.tile_pool(name="tpsum", bufs=2, space="PSUM") as tpool,
            tc.tile_pool(name="dram", bufs=1, space="DRAM") as dpool,
            tc.tile_pool(name="dbounce", bufs=8, space="DRAM") as dbpool,
        ):
            # ---- Phase 0: weights + X^T distribution ----
            whh_sb, wih_sb, bias_sb = {}, {}, {}
            for d in dirs:
                whh_sb[d] = cpool.tile([128, 8 * 512], F32R, tag=f"whh{d}", name=f"whh_sb_{d}")
                nc.sync.dma_start(
                    whh_sb[d][:].rearrange("p (c n) -> p c n", n=512),
                    whh_in[d][:].rearrange("(c p) n -> p c n", p=128),
                )
                wih_sb[d] = cpool.tile([128, 4 * 512], F32R, tag=f"wih{d}", name=f"wih_sb_{d}")
                nc.sync.dma_start(
                    wih_sb[d][:].rearrange("p (c n) -> p c n", n=512),
                    wih_in[d][:].rearrange("(c p) n -> p c n", p=128),
                )
                bias_sb[d] = cpool.tile([1, 512], F32R, tag=f"bias{d}", name=f"bias_sb_{d}")
                nc.sync.dma_start(bias_sb[d][:], bias_in[d][:])
            ones_sb = cpool.tile([1, 128], F32R, tag="ones")
            nc.sync.dma_start(ones_sb[:], ones_in[:])
            ident_sb = cpool.tile([128, 128], F32R, tag="ident")
            nc.sync.dma_start(ident_sb[:], ident_in[:])

            xt_bounce = dpool.tile([IN, TS * B], F32R, tag="xtb")
            nc.sync.dma_start(xt_bounce[:], xt_in[:])
            xt_full = dpool.tile([NC * IN, TS * B], F32R, tag="xtf")
            nc.gpsimd.collective_compute(
                "AllGather", mybir.AluOpType.bypass,
                replica_groups=[list(range(NC))],
                ins=[xt_bounce.opt()], outs=[xt_full.opt()],
            )

            ht = {}
            ct = {}
            for d in dirs:
                ht[d] = hpool.tile([128, H], F32R, tag=f"ht{d}", name=f"ht0_{d}")
                nc.sync.dma_start(ht[d][:], h0t_in[d][:])
                ct[d] = spool.tile([128, 128], F32R, tag=f"c{d}", name=f"ct0_{d}")
                nc.sync.dma_start(ct[d][:], c0_in[d][:])

            # ---- Recurrence ----
            hn_last = {}
            hbin = dbpool.tile([128, 256], F32R, tag="hbin", name="hbin0")
            for rep in range(reps):
                for t in range(t_steps):
                    for d in dirs:
                        td = t if d == "f" else (T - 1 - t)
                        r, tl = divmod(td, TS)

                        xts = xpool.tile([128, 512], F32R, tag=f"x{d}")
                        nc.sync.dma_start(
                            xts[:].rearrange("p (c b) -> p c b", b=128),
                            xt_full[r * IN:(r + 1) * IN, tl * B:(tl + 1) * B]
                            .rearrange("(c p) b -> p c b", p=128),
                        )

                        # gate order [f i o g]: bias (K=1) + 4 x-chunks +
                        # 8 h-chunks accumulate into one PSUM bank
                        ps = pspool.tile([128, 512], F32, tag="ps")
                        nc.tensor.matmul(ps[:], ones_sb[0:1, :], bias_sb[d][0:1, :],
                                         start=True, stop=False)
                        for c in range(4):
                            nc.tensor.matmul(
                                ps[:], xts[:, c * 128:(c + 1) * 128],
                                wih_sb[d][:, c * 512:(c + 1) * 512],
                                start=False, stop=False)
                        for c in range(8):
                            nc.tensor.matmul(
                                ps[:], ht[d][:, c * 128:(c + 1) * 128],
                                whh_sb[d][:, c * 512:(c + 1) * 512],
                                start=False, stop=(c == 7))

                        # g layout [f i o g]: one sigmoid over f,i,o + one tanh
                        g = gpool.tile([128, 512], F32R, tag=f"g{d}")
                        nc.scalar.activation(g[:, 0:384], ps[:, 0:384], AF.Sigmoid)
                        nc.scalar.activation(g[:, 384:512], ps[:, 384:512], AF.Tanh)

                        t1 = spool.tile([128, 128], F32R, tag=f"t1{d}")
                        nc.vector.tensor_mul(t1[:], g[:, 0:128], ct[d][:])
                        t2 = spool.tile([128, 128], F32R, tag=f"t2{d}")
                        nc.vector.tensor_mul(t2[:], g[:, 128:256], g[:, 384:512])
                        cn = spool.tile([128, 128], F32R, tag=f"c{d}")
                        nc.vector.tensor_add(cn[:], t1[:], t2[:])
                        th = spool.tile([128, 128], F32R, tag=f"th{d}")
                        nc.scalar.activation(th[:], cn[:], AF.Tanh)
                        hn = spool.tile([128, 128], F32R, tag=f"hn{d}")
                        nc.vector.tensor_mul(hn[:], g[:, 256:384], th[:])
                        ct[d] = cn

                        # y[t, :, dir*128 : +128] <- h chunk
                        off = 0 if d == "f" else 128
                        nc.sync.dma_start(y_out[td, :, off:off + 128], hn[:])

                        # transpose h chunk into this step's shared bounce half
                        tp = tpool.tile([128, 128], F32R, tag="tp")
                        nc.tensor.transpose(tp[:], hn[:], ident_sb[:])
                        htc = spool.tile([128, 128], F32R, tag=f"htc{d}")
                        nc.vector.tensor_copy(htc[:], tp[:])
                        hoff = 0 if d == "f" else 128
                        nc.sync.dma_start(hbin[:, hoff:hoff + 128], htc[:])
                        hn_last[d] = hn

                    # one AllGather carries both directions' h^T chunks
                    hbout = dbpool.tile([NC * 128, 256], F32R, tag="hbout", name="hbout")
                    nc.gpsimd.collective_compute(
                        "AllGather", mybir.AluOpType.bypass,
                        replica_groups=[list(range(NC))],
                        ins=[hbin.opt()], outs=[hbout.opt()],
                    )
                    hbin = dbpool.tile([128, 256], F32R, tag="hbin", name="hbin_n")
                    for d in dirs:
                        hoff = 0 if d == "f" else 128
                        htn = hpool.tile([128, H], F32R, tag=f"ht{d}", name=f"htn_{d}")
                        htn_v = htn[:].rearrange("p (c b) -> p c b", b=128)
                        hbo_v = hbout[:, hoff:hoff + 128].rearrange("(c p) b -> p c b", p=128)
                        nc.sync.dma_start(htn_v[:, 0:4, :], hbo_v[:, 0:4, :])
                        nc.sync.dma_start(htn_v[:, 4:8, :], hbo_v[:, 4:8, :])
                        ht[d] = htn

                # final states: fwd last step / rev last processed (t=0)
                nc.sync.dma_start(state_out[0], hn_last["f"][:])
                nc.sync.dma_start(state_out[1], ct["f"][:])
                nc.sync.dma_start(state_out[2], hn_last["r"][:])
                nc.sync.dma_start(state_out[3], ct["r"][:])

    nc.compile()
    return nc


def _prep_inputs(x, h0_fwd, c0_fwd, h0_rev, c0_rev,
                 W_ih_f, W_hh_f, b_ih_f, b_hh_f,
                 W_ih_r, W_hh_r, b_ih_r, b_hh_r):
    f32 = np.float32
    x = np.ascontiguousarray(x, f32)
    xt_all = np.ascontiguousarray(x.reshape(T * B, IN).T)  # [IN, T*B]
    per_dir = {
        "f": (W_ih_f, W_hh_f, b_ih_f, b_hh_f, h0_fwd, c0_fwd),
        "r": (W_ih_r, W_hh_r, b_ih_r, b_hh_r, h0_rev, c0_rev),
    }
    ones = np.ones((1, 128), f32)
    ident = np.eye(128, dtype=f32)
    in_maps = []
    for j in range(NC):
        idx = np.concatenate([
            np.arange(128) + 1024 + j * 128,   # f
            np.arange(128) + 0 + j * 128,      # i
            np.arange(128) + 3072 + j * 128,   # o
            np.arange(128) + 2048 + j * 128,   # g
        ])
        m = {
            "xt": np.ascontiguousarray(xt_all[:, j * TS * B:(j + 1) * TS * B]),
            "ones": ones, "ident": ident,
        }
        for d in dirs_tuple():
            Wih, Whh, bih, bhh, h0, c0 = per_dir[d]
            m[f"whh_{d}"] = np.ascontiguousarray(np.asarray(Whh, f32)[idx, :].T)
            m[f"wih_{d}"] = np.ascontiguousarray(np.asarray(Wih, f32)[idx, :].T)
            m[f"bias_{d}"] = np.ascontiguousarray(
                (np.asarray(bih, f32) + np.asarray(bhh, f32))[idx][None, :])
            m[f"h0t_{d}"] = np.ascontiguousarray(
                np.asarray(h0, f32).reshape(B, 8, 128).transpose(2, 1, 0).reshape(128, H))
            m[f"c0_{d}"] = np.ascontiguousarray(np.asarray(c0, f32)[:, j * 128:(j + 1) * 128])
        in_maps.append(m)
    return in_maps


def dirs_tuple():
    return ("f", "r")


def run(inputs, reps=1):
    key = reps
    if key not in _cache:
        _cache[key] = build(reps)
    nc = _cache[key]
    in_maps = _prep_inputs(**inputs)
    res = bass_utils.run_bass_kernel_spmd(
        nc, in_maps, core_ids=list(range(NC)), trace=False)
    return res.results


def kernel(**inputs):
    results = run(inputs, reps=1)
    f32 = np.float32
    y = np.empty((T, B, 2 * H), f32)
    hf = np.empty((B, H), f32)
    cf = np.empty((B, H), f32)
    hr = np.empty((B, H), f32)
    cr = np.empty((B, H), f32)
    for j in range(NC):
        yj = results[j]["y"]
        y[:, :, j * 128:(j + 1) * 128] = yj[:, :, 0:128]
        y[:, :, H + j * 128:H + (j + 1) * 128] = yj[:, :, 128:256]
        st = results[j]["state"]
        hf[:, j * 128:(j + 1) * 128] = st[0]
        cf[:, j * 128:(j + 1) * 128] = st[1]
        hr[:, j * 128:(j + 1) * 128] = st[2]
        cr[:, j * 128:(j + 1) * 128] = st[3]
    hidden = np.stack([hf, cf])
    cell = np.stack([hr, cr])
    return y, (hidden, cell)


# revision 10
# speedup vs baseline: 7.4836x; 1.0796x over previous
"""Bidirectional LSTM layer on 8 Trainium2 NeuronCores.

Strategy (spec: T=256, B=128, IN=512, H=1024):
  - 8-way tensor-parallel over the 4H gate dim: core j owns gate rows
    {g*1024 + j*128 .. +128} for g in (f, i, g, o) [reordered so one
    sigmoid covers f+i], i.e. a 512-wide gate slice, and therefore owns
    h/c columns j*128..(j+1)*128.
  - Per timestep: PSUM[b=128, 512] = bias (K=1 matmul) + x_t^T-chunks @
    Wih^T-slice + h_t^T-chunks @ Whh^T-slice, all f32r (full-rate fp32).
    Activations read PSUM; cell update on DVE; h chunk is PE-transposed
    and AllGathered (per direction) into the next step's stationary h^T.
  - fwd and rev directions are independent chains, staggered so PE works
    on one while the other's AllGather is in flight.
  - X^T is distributed once at start: each core transposes nothing; the
    host passes its T-slice of X^T and one AllGather replicates it.
"""
import sys
sys.path.insert(0, "/opt/trn_rl_repo")
import numpy as np
import concourse.bacc as bacc
import concourse.mybir as mybir
from concourse import tile
from concourse import bass_utils

T, B, IN, H = 256, 128, 512, 1024
NC = 8
TS = T // NC            # timesteps per core in the X^T shard
F32R = mybir.dt.float32r
F32 = mybir.dt.float32
AF = mybir.ActivationFunctionType

_cache = {}


def build(reps=1, t_steps=T):
    nc = bacc.Bacc("TRN2", target_bir_lowering=False, debug=False, num_devices=NC)

    xt_in = nc.dram_tensor("xt", [IN, TS * B], F32R, kind="ExternalInput")
    dirs = ("f", "r")
    whh_in = {d: nc.dram_tensor(f"whh_{d}", [H, 512], F32R, kind="ExternalInput") for d in dirs}
    wih_in = {d: nc.dram_tensor(f"wih_{d}", [IN, 512], F32R, kind="ExternalInput") for d in dirs}
    bias_in = {d: nc.dram_tensor(f"bias_{d}", [1, 512], F32R, kind="ExternalInput") for d in dirs}
    h0t_in = {d: nc.dram_tensor(f"h0t_{d}", [128, H], F32R, kind="ExternalInput") for d in dirs}
    c0_in = {d: nc.dram_tensor(f"c0_{d}", [128, 128], F32R, kind="ExternalInput") for d in dirs}
    ones_in = nc.dram_tensor("ones", [1, 128], F32R, kind="ExternalInput")
    ident_in = nc.dram_tensor("ident", [128, 128], F32R, kind="ExternalInput")

    y_out = nc.dram_tensor("y", [T, B, 256], F32R, kind="ExternalOutput")
    state_out = nc.dram_tensor("state", [4, 128, 128], F32R, kind="ExternalOutput")

    with tile.TileContext(nc) as tc:
        with (
            tc.tile_pool(name="const", bufs=1) as cpool,
            tc.tile_pool(name="xts", bufs=6) as xpool,
            tc.tile_pool(name="ht", bufs=4) as hpool,
            tc.tile_pool(name="gates", bufs=4) as gpool,
            tc.tile_pool(name="small", bufs=4) as spool,
            tc.tile_pool(name="gpsum", bufs=4, space="PSUM") as pspool,
            tc.tile_pool(name="tpsum", bufs=2, space="PSUM") as tpool,
            tc.tile_pool(name="dram", bufs=1, space="DRAM") as dpool,
            tc.tile_pool(name="dbounce", bufs=8, space="DRAM") as dbpool,
        ):
            # ---- Phase 0: weights + X^T distribution ----
            whh_sb, wih_sb, bias_sb = {}, {}, {}
            for d in dirs:
                whh_sb[d] = cpool.tile([128, 8 * 512], F32R, tag=f"whh{d}", name=f"whh_sb_{d}")
                nc.sync.dma_start(
                    whh_sb[d][:].rearrange("p (c n) -> p c n", n=512),
                    whh_in[d][:].rearrange("(c p) n -> p c n", p=128),
                )
                wih_sb[d] = cpool.tile([128, 4 * 512], F32R, tag=f"wih{d}", name=f"wih_sb_{d}")
                nc.sync.dma_start(
                    wih_sb[d][:].rearrange("p (c n) -> p c n", n=512),
                    wih_in[d][:].rearrange("(c p) n -> p c n", p=128),
                )
                bias_sb[d] = cpool.tile([1, 512], F32R, tag=f"bias{d}", name=f"bias_sb_{d}")
                nc.sync.dma_start(bias_sb[d][:], bias_in[d][:])
            ones_sb = cpool.tile([1, 128], F32R, tag="ones")
            nc.sync.dma_start(ones_sb[:], ones_in[:])
            ident_sb = cpool.tile([128, 128], F32R, tag="ident")
            nc.sync.dma_start(ident_sb[:], ident_in[:])

            xt_bounce = dpool.tile([IN, TS * B], F32R, tag="xtb")
            nc.sync.dma_start(xt_bounce[:], xt_in[:])
            xt_full = dpool.tile([NC * IN, TS * B], F32R, tag="xtf")
            nc.gpsimd.collective_compute(
                "AllGather", mybir.AluOpType.bypass,
                replica_groups=[list(range(NC))],
                ins=[xt_bounce.opt()], outs=[xt_full.opt()],
            )

            ht = {}
            ct = {}
            for d in dirs:
                ht[d] = hpool.tile([128, H], F32R, tag=f"ht{d}", name=f"ht0_{d}")
                nc.sync.dma_start(ht[d][:], h0t_in[d][:])
                ct[d] = spool.tile([128, 256], F32R, tag=f"c{d}", name=f"ct0_{d}")
                nc.sync.dma_start(ct[d][:, 0:128], c0_in[d][:])

            # ---- Recurrence ----
            hn_last = {}
            xpair = {}
            hbin = dbpool.tile([128, 256], F32R, tag="hbin", name="hbin0")
            for rep in range(reps):
                for t in range(t_steps):
                    htc = spool.tile([128, 256], F32R, tag="htc", name="htc")
                    for d in dirs:
                        td = t if d == "f" else (T - 1 - t)
                        r, tl = divmod(td, TS)

                        # pair-load x^T for two consecutive steps in one DMA;
                        # sbuf free layout (c, s*b) keeps the DMA 3-dim
                        pair_first = (tl % 2 == 0) if d == "f" else (tl % 2 == 1)
                        if pair_first:
                            xts2 = xpool.tile([128, 1024], F32R, tag=f"x{d}", name=f"xts2{d}")
                            tl0 = tl if d == "f" else tl - 1
                            nc.sync.dma_start(
                                xts2[:].rearrange("p (c sb) -> p c sb", sb=256),
                                xt_full[r * IN:(r + 1) * IN, tl0 * B:(tl0 + 2) * B]
                                .rearrange("(c p) sb -> p c sb", p=128),
                            )
                            xpair[d] = xts2
                        sl2 = (tl % 2) * 128
                        xbase = xpair[d]

                        # gate order [f i o g]: bias (K=1) + 4 x-chunks +
                        # 8 h-chunks accumulate into one PSUM bank
                        ps = pspool.tile([128, 512], F32, tag="ps")
                        nc.tensor.matmul(ps[:], ones_sb[0:1, :], bias_sb[d][0:1, :],
                                         start=True, stop=False)
                        for c in range(4):
                            nc.tensor.matmul(
                                ps[:], xbase[:, c * 256 + sl2:c * 256 + sl2 + 128],
                                wih_sb[d][:, c * 512:(c + 1) * 512],
                                start=False, stop=False)
                        for c in range(8):
                            nc.tensor.matmul(
                                ps[:], ht[d][:, c * 128:(c + 1) * 128],
                                whh_sb[d][:, c * 512:(c + 1) * 512],
                                start=False, stop=(c == 7))

                        # g layout [f i o]; tanh(g-gate) lands in cg[:,128:256]
                        # next to c_old so one [128,256] mul yields f'c | i'g'
                        g = gpool.tile([128, 384], F32R, tag=f"g{d}")
                        nc.scalar.activation(g[:, 0:384], ps[:, 0:384], AF.Sigmoid)
                        cg = ct[d]
                        nc.scalar.activation(cg[:, 128:256], ps[:, 384:512], AF.Tanh)
                        y2 = spool.tile([128, 256], F32R, tag=f"y2{d}")
                        nc.vector.tensor_mul(y2[:], g[:, 0:256], cg[:])
                        cgn = spool.tile([128, 256], F32R, tag=f"c{d}", name=f"cgn{d}")
                        nc.vector.tensor_add(cgn[:, 0:128], y2[:, 0:128], y2[:, 128:256])
                        th = spool.tile([128, 128], F32R, tag=f"th{d}")
                        nc.scalar.activation(th[:], cgn[:, 0:128], AF.Tanh)
                        hn = spool.tile([128, 128], F32R, tag=f"hn{d}")
                        nc.vector.tensor_mul(hn[:], g[:, 256:384], th[:])
                        ct[d] = cgn

                        # y[t, :, dir*128 : +128] <- h chunk
                        off = 0 if d == "f" else 128
                        nc.sync.dma_start(y_out[td, :, off:off + 128], hn[:])

                        # transpose h chunk into this step's shared bounce half
                        tp = tpool.tile([128, 128], F32R, tag="tp")
                        nc.tensor.transpose(tp[:], hn[:], ident_sb[:])
                        hoff = 0 if d == "f" else 128
                        nc.vector.tensor_copy(htc[:, hoff:hoff + 128], tp[:])
                        hn_last[d] = hn

                    nc.sync.dma_start(hbin[:], htc[:])
                    # one AllGather carries both directions' h^T chunks
                    hbout = dbpool.tile([NC * 128, 256], F32R, tag="hbout", name="hbout")
                    nc.gpsimd.collective_compute(
                        "AllGather", mybir.AluOpType.bypass,
                        replica_groups=[list(range(NC))],
                        ins=[hbin.opt()], outs=[hbout.opt()],
                    )
                    hbin = dbpool.tile([128, 256], F32R, tag="hbin", name="hbin_n")
                    for d in dirs:
                        hoff = 0 if d == "f" else 128
                        htn = hpool.tile([128, H], F32R, tag=f"ht{d}", name=f"htn_{d}")
                        nc.sync.dma_start(
                            htn[:].rearrange("p (c b) -> p c b", b=128),
                            hbout[:, hoff:hoff + 128].rearrange("(c p) b -> p c b", p=128))
                        ht[d] = htn

                # final states: fwd last step / rev last processed (t=0)
                nc.sync.dma_start(state_out[0], hn_last["f"][:])
                nc.sync.dma_start(state_out[1], ct["f"][:, 0:128])
                nc.sync.dma_start(state_out[2], hn_last["r"][:])
                nc.sync.dma_start(state_out[3], ct["r"][:, 0:128])

    nc.compile()
    return nc


def _prep_inputs(x, h0_fwd, c0_fwd, h0_rev, c0_rev,
                 W_ih_f, W_hh_f, b_ih_f, b_hh_f,
                 W_ih_r, W_hh_r, b_ih_r, b_hh_r):
    f32 = np.float32
    x = np.ascontiguousarray(x, f32)
    xt_all = np.ascontiguousarray(x.reshape(T * B, IN).T)  # [IN, T*B]
    per_dir = {
        "f": (W_ih_f, W_hh_f, b_ih_f, b_hh_f, h0_fwd, c0_fwd),
        "r": (W_ih_r, W_hh_r, b_ih_r, b_hh_r, h0_rev, c0_rev),
    }
    ones = np.ones((1, 128), f32)
    ident = np.eye(128, dtype=f32)
    in_maps = []
    for j in range(NC):
        idx = np.concatenate([
            np.arange(128) + 1024 + j * 128,   # f
            np.arange(128) + 0 + j * 128,      # i
            np.arange(128) + 3072 + j * 128,   # o
            np.arange(128) + 2048 + j * 128,   # g
        ])
        m = {
            "xt": np.ascontiguousarray(xt_all[:, j * TS * B:(j + 1) * TS * B]),
            "ones": ones, "ident": ident,
        }
        for d in dirs_tuple():
            Wih, Whh, bih, bhh, h0, c0 = per_dir[d]
            m[f"whh_{d}"] = np.ascontiguousarray(np.asarray(Whh, f32)[idx, :].T)
            m[f"wih_{d}"] = np.ascontiguousarray(np.asarray(Wih, f32)[idx, :].T)
            m[f"bias_{d}"] = np.ascontiguousarray(
                (np.asarray(bih, f32) + np.asarray(bhh, f32))[idx][None, :])
            m[f"h0t_{d}"] = np.ascontiguousarray(
                np.asarray(h0, f32).reshape(B, 8, 128).transpose(2, 1, 0).reshape(128, H))
            m[f"c0_{d}"] = np.ascontiguousarray(np.asarray(c0, f32)[:, j * 128:(j + 1) * 128])
        in_maps.append(m)
    return in_maps


def dirs_tuple():
    return ("f", "r")


def run(inputs, reps=1):
    key = reps
    if key not in _cache:
        _cache[key] = build(reps)
    nc = _cache[key]
    in_maps = _prep_inputs(**inputs)
    res = bass_utils.run_bass_kernel_spmd(
        nc, in_maps, core_ids=list(range(NC)), trace=False)
    return res.results


def kernel(**inputs):
    results = run(inputs, reps=1)
    f32 = np.float32
    y = np.empty((T, B, 2 * H), f32)
    hf = np.empty((B, H), f32)
    cf = np.empty((B, H), f32)
    hr = np.empty((B, H), f32)
    cr = np.empty((B, H), f32)
    for j in range(NC):
        yj = results[j]["y"]
        y[:, :, j * 128:(j + 1) * 128] = yj[:, :, 0:128]
        y[:, :, H + j * 128:H + (j + 1) * 128] = yj[:, :, 128:256]
        st = results[j]["state"]
        hf[:, j * 128:(j + 1) * 128] = st[0]
        cf[:, j * 128:(j + 1) * 128] = st[1]
        hr[:, j * 128:(j + 1) * 128] = st[2]
        cr[:, j * 128:(j + 1) * 128] = st[3]
    hidden = np.stack([hf, cf])
    cell = np.stack([hr, cr])
    return y, (hidden, cell)


# revision 11
# speedup vs baseline: 9.5987x; 1.2826x over previous
"""Bidirectional LSTM layer on 8 Trainium2 NeuronCores.

Strategy (spec: T=256, B=128, IN=512, H=1024):
  - 8-way tensor-parallel over the 4H gate dim: core j owns gate rows
    {g*1024 + j*128 .. +128} for g in (f, i, g, o) [reordered so one
    sigmoid covers f+i], i.e. a 512-wide gate slice, and therefore owns
    h/c columns j*128..(j+1)*128.
  - Per timestep: PSUM[b=128, 512] = bias (K=1 matmul) + x_t^T-chunks @
    Wih^T-slice + h_t^T-chunks @ Whh^T-slice, all f32r (full-rate fp32).
    Activations read PSUM; cell update on DVE; h chunk is PE-transposed
    and AllGathered (per direction) into the next step's stationary h^T.
  - fwd and rev directions are independent chains, staggered so PE works
    on one while the other's AllGather is in flight.
  - X^T is distributed once at start: each core transposes nothing; the
    host passes its T-slice of X^T and one AllGather replicates it.
"""
import sys
sys.path.insert(0, "/opt/trn_rl_repo")
import numpy as np
import concourse.bacc as bacc
import concourse.mybir as mybir
from concourse import tile
from concourse import bass_utils

T, B, IN, H = 256, 128, 512, 1024
NC = 8
TS = T // NC            # timesteps per core in the X^T shard
F32R = mybir.dt.float32r
F32 = mybir.dt.float32
AF = mybir.ActivationFunctionType

_cache = {}


def build(reps=1, t_steps=T):
    nc = bacc.Bacc("TRN2", target_bir_lowering=False, debug=False, num_devices=NC)

    xt_in = nc.dram_tensor("xt", [IN, TS * B], F32R, kind="ExternalInput")
    dirs = ("f", "r")
    whh_in = {d: nc.dram_tensor(f"whh_{d}", [H, 512], F32R, kind="ExternalInput") for d in dirs}
    wih_in = {d: nc.dram_tensor(f"wih_{d}", [IN, 512], F32R, kind="ExternalInput") for d in dirs}
    bias_in = {d: nc.dram_tensor(f"bias_{d}", [1, 512], F32R, kind="ExternalInput") for d in dirs}
    h0t_in = {d: nc.dram_tensor(f"h0t_{d}", [128, H], F32R, kind="ExternalInput") for d in dirs}
    c0_in = {d: nc.dram_tensor(f"c0_{d}", [128, 128], F32R, kind="ExternalInput") for d in dirs}
    ones_in = nc.dram_tensor("ones", [1, 128], F32R, kind="ExternalInput")
    ident_in = nc.dram_tensor("ident", [128, 128], F32R, kind="ExternalInput")

    y_out = nc.dram_tensor("y", [T, B, 256], F32R, kind="ExternalOutput")
    state_out = nc.dram_tensor("state", [4, 128, 128], F32R, kind="ExternalOutput")

    with tile.TileContext(nc) as tc:
        with (
            tc.tile_pool(name="const", bufs=1) as cpool,
            tc.tile_pool(name="xts", bufs=6) as xpool,
            tc.tile_pool(name="ht", bufs=4) as hpool,
            tc.tile_pool(name="gates", bufs=4) as gpool,
            tc.tile_pool(name="small", bufs=4) as spool,
            tc.tile_pool(name="gpsum", bufs=4, space="PSUM") as pspool,
            tc.tile_pool(name="tpsum", bufs=2, space="PSUM") as tpool,
            tc.tile_pool(name="dram", bufs=1, space="DRAM") as dpool,
            tc.tile_pool(name="dbounce", bufs=8, space="DRAM") as dbpool,
        ):
            # ---- Phase 0: weights + X^T distribution ----
            whh_sb, wih_sb, bias_sb = {}, {}, {}
            for d in dirs:
                whh_sb[d] = cpool.tile([128, 8 * 512], F32R, tag=f"whh{d}", name=f"whh_sb_{d}")
                nc.sync.dma_start(
                    whh_sb[d][:].rearrange("p (c n) -> p c n", n=512),
                    whh_in[d][:].rearrange("(c p) n -> p c n", p=128),
                )
                wih_sb[d] = cpool.tile([128, 4 * 512], F32R, tag=f"wih{d}", name=f"wih_sb_{d}")
                nc.sync.dma_start(
                    wih_sb[d][:].rearrange("p (c n) -> p c n", n=512),
                    wih_in[d][:].rearrange("(c p) n -> p c n", p=128),
                )
                bias_sb[d] = cpool.tile([1, 512], F32R, tag=f"bias{d}", name=f"bias_sb_{d}")
                nc.sync.dma_start(bias_sb[d][:], bias_in[d][:])
            ones_sb = cpool.tile([1, 128], F32R, tag="ones")
            nc.sync.dma_start(ones_sb[:], ones_in[:])
            ident_sb = cpool.tile([128, 128], F32R, tag="ident")
            nc.sync.dma_start(ident_sb[:], ident_in[:])

            xt_bounce = dpool.tile([IN, TS * B], F32R, tag="xtb")
            nc.sync.dma_start(xt_bounce[:], xt_in[:])
            xt_full = dpool.tile([NC * IN, TS * B], F32R, tag="xtf")
            nc.gpsimd.collective_compute(
                "AllGather", mybir.AluOpType.bypass,
                replica_groups=[list(range(NC))],
                ins=[xt_bounce.opt()], outs=[xt_full.opt()],
            )

            ht = {}
            for d in dirs:
                ht[d] = hpool.tile([128, H], F32R, tag=f"ht{d}", name=f"ht0_{d}")
                nc.sync.dma_start(ht[d][:], h0t_in[d][:])
            # shared cell tile: [c_f | g'_f | c_r | g'_r] in 4x128 quarters
            ct2 = spool.tile([128, 512], F32R, tag="c", name="ct2_0")
            nc.sync.dma_start(ct2[:, 0:128], c0_in["f"][:])
            nc.sync.dma_start(ct2[:, 256:384], c0_in["r"][:])

            # ---- Recurrence ----
            hn_last = {}
            xpair = {}
            hbin = dbpool.tile([128, 256], F32R, tag="hbin", name="hbin0")
            for rep in range(reps):
                for t in range(t_steps):
                    htc = spool.tile([128, 256], F32R, tag="htc", name="htc")
                    g2 = gpool.tile([128, 768], F32R, tag="g", name="g2")
                    for d in dirs:
                        td = t if d == "f" else (T - 1 - t)
                        r, tl = divmod(td, TS)

                        # pair-load x^T for two consecutive steps in one DMA;
                        # sbuf free layout (c, s*b) keeps the DMA 3-dim
                        pair_first = (tl % 2 == 0) if d == "f" else (tl % 2 == 1)
                        if pair_first:
                            xts2 = xpool.tile([128, 1024], F32R, tag=f"x{d}", name=f"xts2{d}")
                            tl0 = tl if d == "f" else tl - 1
                            nc.sync.dma_start(
                                xts2[:].rearrange("p (c sb) -> p c sb", sb=256),
                                xt_full[r * IN:(r + 1) * IN, tl0 * B:(tl0 + 2) * B]
                                .rearrange("(c p) sb -> p c sb", p=128),
                            )
                            xpair[d] = xts2
                        sl2 = (tl % 2) * 128
                        xbase = xpair[d]

                        # gate order [f i o g]: bias (K=1) + 4 x-chunks +
                        # 8 h-chunks accumulate into one PSUM bank
                        ps = pspool.tile([128, 512], F32, tag="ps")
                        nc.tensor.matmul(ps[:], ones_sb[0:1, :], bias_sb[d][0:1, :],
                                         start=True, stop=False)
                        for c in range(4):
                            nc.tensor.matmul(
                                ps[:], xbase[:, c * 256 + sl2:c * 256 + sl2 + 128],
                                wih_sb[d][:, c * 512:(c + 1) * 512],
                                start=False, stop=False)
                        for c in range(8):
                            nc.tensor.matmul(
                                ps[:], ht[d][:, c * 128:(c + 1) * 128],
                                whh_sb[d][:, c * 512:(c + 1) * 512],
                                start=False, stop=(c == 7))

                        # per-dir (psum-gated): sigmoid f,i,o into g2 half;
                        # tanh(g-gate) lands next to this dir's c in ct2
                        go = 0 if d == "f" else 384
                        co = 0 if d == "f" else 256
                        nc.scalar.activation(g2[:, go:go + 384], ps[:, 0:384], AF.Sigmoid)
                        nc.scalar.activation(ct2[:, co + 128:co + 256], ps[:, 384:512], AF.Tanh)

                    # fused cell update for BOTH directions via strided APs
                    g2v = g2[:].rearrange("p (d q) -> p d q", q=384)
                    c2v = ct2[:].rearrange("p (d q) -> p d q", q=256)
                    y4 = spool.tile([128, 512], F32R, tag="y2f", name="y4")
                    y4v = y4[:].rearrange("p (d q) -> p d q", q=256)
                    nc.vector.tensor_mul(y4v, g2v[:, :, 0:256], c2v)
                    ctn = spool.tile([128, 512], F32R, tag="c", name="ctn")
                    ctnv = ctn[:].rearrange("p (d q) -> p d q", q=256)
                    nc.vector.tensor_add(ctnv[:, :, 0:128], y4v[:, :, 0:128], y4v[:, :, 128:256])
                    th2 = spool.tile([128, 256], F32R, tag="th", name="th2")
                    th2v = th2[:].rearrange("p (d q) -> p d q", q=128)
                    nc.scalar.activation(th2v, ctnv[:, :, 0:128], AF.Tanh)
                    hn2 = spool.tile([128, 256], F32R, tag="hn", name="hn2")
                    nc.vector.tensor_mul(hn2[:].rearrange("p (d q) -> p d q", q=128),
                                         g2v[:, :, 256:384], th2v)
                    ct2 = ctn

                    for d in dirs:
                        td = t if d == "f" else (T - 1 - t)
                        off = 0 if d == "f" else 128
                        nc.sync.dma_start(y_out[td, :, off:off + 128], hn2[:, off:off + 128])
                        tp = tpool.tile([128, 128], F32R, tag="tp", name="tp")
                        nc.tensor.transpose(tp[:], hn2[:, off:off + 128], ident_sb[:])
                        nc.vector.tensor_copy(htc[:, off:off + 128], tp[:])
                    hn_last["t"] = hn2

                    nc.sync.dma_start(hbin[:], htc[:])
                    # one AllGather carries both directions' h^T chunks
                    hbout = dbpool.tile([NC * 128, 256], F32R, tag="hbout", name="hbout")
                    nc.gpsimd.collective_compute(
                        "AllGather", mybir.AluOpType.bypass,
                        replica_groups=[list(range(NC))],
                        ins=[hbin.opt()], outs=[hbout.opt()],
                    )
                    hbin = dbpool.tile([128, 256], F32R, tag="hbin", name="hbin_n")
                    for d in dirs:
                        hoff = 0 if d == "f" else 128
                        htn = hpool.tile([128, H], F32R, tag=f"ht{d}", name=f"htn_{d}")
                        nc.sync.dma_start(
                            htn[:].rearrange("p (c b) -> p c b", b=128),
                            hbout[:, hoff:hoff + 128].rearrange("(c p) b -> p c b", p=128))
                        ht[d] = htn

                # final states: fwd last step / rev last processed (t=0)
                nc.sync.dma_start(state_out[0], hn_last["t"][:, 0:128])
                nc.sync.dma_start(state_out[1], ct2[:, 0:128])
                nc.sync.dma_start(state_out[2], hn_last["t"][:, 128:256])
                nc.sync.dma_start(state_out[3], ct2[:, 256:384])

    nc.compile()
    return nc


def _prep_inputs(x, h0_fwd, c0_fwd, h0_rev, c0_rev,
                 W_ih_f, W_hh_f, b_ih_f, b_hh_f,
                 W_ih_r, W_hh_r, b_ih_r, b_hh_r):
    f32 = np.float32
    x = np.ascontiguousarray(x, f32)
    xt_all = np.ascontiguousarray(x.reshape(T * B, IN).T)  # [IN, T*B]
    per_dir = {
        "f": (W_ih_f, W_hh_f, b_ih_f, b_hh_f, h0_fwd, c0_fwd),
        "r": (W_ih_r, W_hh_r, b_ih_r, b_hh_r, h0_rev, c0_rev),
    }
    ones = np.ones((1, 128), f32)
    ident = np.eye(128, dtype=f32)
    in_maps = []
    for j in range(NC):
        idx = np.concatenate([
            np.arange(128) + 1024 + j * 128,   # f
            np.arange(128) + 0 + j * 128,      # i
            np.arange(128) + 3072 + j * 128,   # o
            np.arange(128) + 2048 + j * 128,   # g
        ])
        m = {
            "xt": np.ascontiguousarray(xt_all[:, j * TS * B:(j + 1) * TS * B]),
            "ones": ones, "ident": ident,
        }
        for d in dirs_tuple():
            Wih, Whh, bih, bhh, h0, c0 = per_dir[d]
            m[f"whh_{d}"] = np.ascontiguousarray(np.asarray(Whh, f32)[idx, :].T)
            m[f"wih_{d}"] = np.ascontiguousarray(np.asarray(Wih, f32)[idx, :].T)
            m[f"bias_{d}"] = np.ascontiguousarray(
                (np.asarray(bih, f32) + np.asarray(bhh, f32))[idx][None, :])
            m[f"h0t_{d}"] = np.ascontiguousarray(
                np.asarray(h0, f32).reshape(B, 8, 128).transpose(2, 1, 0).reshape(128, H))
            m[f"c0_{d}"] = np.ascontiguousarray(np.asarray(c0, f32)[:, j * 128:(j + 1) * 128])
        in_maps.append(m)
    return in_maps


def dirs_tuple():
    return ("f", "r")


def run(inputs, reps=1):
    key = reps
    if key not in _cache:
        _cache[key] = build(reps)
    nc = _cache[key]
    in_maps = _prep_inputs(**inputs)
    res = bass_utils.run_bass_kernel_spmd(
        nc, in_maps, core_ids=list(range(NC)), trace=False)
    return res.results


def kernel(**inputs):
    results = run(inputs, reps=1)
    f32 = np.float32
    y = np.empty((T, B, 2 * H), f32)
    hf = np.empty((B, H), f32)
    cf = np.empty((B, H), f32)
    hr = np.empty((B, H), f32)
    cr = np.empty((B, H), f32)
    for j in range(NC):
        yj = results[j]["y"]
        y[:, :, j * 128:(j + 1) * 128] = yj[:, :, 0:128]
        y[:, :, H + j * 128:H + (j + 1) * 128] = yj[:, :, 128:256]
        st = results[j]["state"]
        hf[:, j * 128:(j + 1) * 128] = st[0]
        cf[:, j * 128:(j + 1) * 128] = st[1]
        hr[:, j * 128:(j + 1) * 128] = st[2]
        cr[:, j * 128:(j + 1) * 128] = st[3]
    hidden = np.stack([hf, cf])
    cell = np.stack([hr, cr])
    return y, (hidden, cell)
